# revision 8
# baseline (speedup 1.0000x reference)
"""Causal self-attention with RoPE on 8 TRN2 NeuronCores.

Sharding: tensor-parallel over heads (H=8 -> 1 head per core).

Head dims are pair-reordered on the host (evens then odds per 64-dim
half) so that RoPE's pair-swap becomes two contiguous 32-row half-swaps
and the cos/sin tables become 32-row periodic:
 - qks comes from 4 SBUF->SBUF half-swap DMAs on the bf16 qk copy
   instead of a second projection matmul group (saves 4 PE matmuls/blk).
 - cc/ss ship compact ([32,T]/[64,T]) and expand on-chip with 3 DMAs,
   cutting input HBM traffic by 1.25MB so early blocks arrive in time.

Pipelined loop emits A(tb+1) projections before B(tb) attention so the
Tile scheduler overlaps them:
 - A(tb): qk matmuls -> bf16 copy -> swap DMAs -> RoPE combine on DVE
   (all-bf16 tensor ops run in 2x mode); v in (t,d) layout; krqr
   half-swap via SBUF DMA for quadrant pairing.
 - B(ib): S^T chunk pairs as concurrent K=64 quadrant matmuls, emitted
   in groups of two pairs so LDWEIGHTS of the next pair can pull ahead;
   exp(S^T/8) one instruction per pair, ~2/3 on ACT (true exp), ~1/3 on
   DVE via a Schraudolph bf16 bit-trick; causal diag masking via gpsimd
   affine_select; PV matmuls with a ones-column for the denominator.
 - c_proj runs as concurrent K=64 quadrant pairs (wp duplicated on host
   to rows 64:128; y^T duplicated on-chip via an SBUF->SBUF DMA). The
   final block borrows free S^T PSUM banks so its four quadrants run
   without write-after-read stalls, and streams out per-pair.
 - A short burst of warm-up matmuls keeps the PE HAM clock-gate warm
   while the first input DMAs land; inputs are issued on sync in
   consumption order with distance-2 prefetch so no data-dependent DMA
   head-of-line-blocks them.
 - out_u + cs shipped to host; host divides and sums over heads.
"""
import sys

sys.path.insert(0, "/opt/trn_rl_repo")

import numpy as np
import ml_dtypes

import concourse.bass as bass
import concourse.mybir as mybir
import concourse.tile as tile
from concourse.bass_utils import run_bass_kernel_spmd

B, T, C, H = 1, 4096, 512, 8
HS = C // H  # 64
NCORES = 8
TB = 512           # t-block width for projections / i-block width for attention
NTB = T // TB      # 8
JC = 128           # j-chunk width
NJC = T // JC      # 32

SCH_A = float(16.0 * np.log2(np.e))   # Schraudolph slope (includes /8 scale)
SCH_B = 16250.375                     # RNE-optimal offset (bf16 bit space)
DVE_EXP_FRAC = 3                      # every 3rd pair -> DVE exp
N_WARMUP = 12                         # PE warm-up matmuls during input DMA

_ctr = [0]


def _legalize_waits(nc):
    """This walrus build accepts at most one sem-wait command per hw
    instruction; move extra waits onto same-engine NoOps inserted before."""
    for f in nc.m.functions:
        for bb in f.blocks:
            insts = bb.instructions
            out = []
            for inst in insts:
                si = inst.sync_info
                if si is not None and len(si.on_wait) > 1:
                    waits = list(si.on_wait)
                    for w in waits[:-1]:
                        _ctr[0] += 1
                        nop = mybir.InstNoOp(name=f"I-waitsplit-{_ctr[0]}")
                        nop.engine = inst.engine
                        nop.sync_info = mybir.SyncInfo(on_wait=[w], on_update=[])
                        out.append(nop)
                    inst.sync_info = mybir.SyncInfo(
                        on_wait=[waits[-1]], on_update=list(si.on_update)
                    )
                out.append(inst)
            insts[:] = out
    return nc


def _build_nc():
    nc = bass.Bass()
    f32 = mybir.dt.float32
    bf16 = mybir.dt.bfloat16
    u16 = mybir.dt.uint16

    xt_in = nc.declare_dram_parameter("xt", [128, NTB, 4, TB], bf16, isOutput=False)
    wqk_in = nc.declare_dram_parameter("wqk", [128, 4, 128], bf16, isOutput=False)
    wv_in = nc.declare_dram_parameter("wv", [128, 4, HS], bf16, isOutput=False)
    wp_in = nc.declare_dram_parameter("wp", [128, C], bf16, isOutput=False)
    ccc_in = nc.declare_dram_parameter("ccc", [32, T], bf16, isOutput=False)
    ssc_in = nc.declare_dram_parameter("ssc", [64, T], bf16, isOutput=False)
    out_u = nc.declare_dram_parameter("out_u", [128, NTB, 4, TB], bf16, isOutput=True)
    cs_out = nc.declare_dram_parameter("cs", [1, T], bf16, isOutput=True)

    Exp = mybir.ActivationFunctionType.Exp
    Mult = mybir.AluOpType.mult
    Add = mybir.AluOpType.add

    with tile.TileContext(nc) as tc:
        with (
            tc.tile_pool(name="big", bufs=1) as big,
            tc.tile_pool(name="ropet", bufs=2) as ropet,
            tc.tile_pool(name="qkb", bufs=2) as qkb,
            tc.tile_pool(name="qksb", bufs=2) as qksb,
            tc.tile_pool(name="ptp", bufs=7) as ptp,
            tc.tile_pool(name="ytsb", bufs=2) as ytsb,
            tc.tile_pool(name="ytdp", bufs=2) as ytdp,
            tc.tile_pool(name="outp", bufs=2) as outp,
            tc.tile_pool(name="mmp", bufs=2, space="PSUM") as mmp,
            tc.tile_pool(name="vpp", bufs=1, space="PSUM") as vpp,
            tc.tile_pool(name="stp", bufs=2, space="PSUM") as stp,
            tc.tile_pool(name="ytp", bufs=1, space="PSUM") as ytp,
        ):
            # ---- PE warm-up on scratch data: keeps the HAM clock-gate
            # busy while the first input DMAs land.
            wsrc = big.tile([128, 128], bf16)
            nc.vector.memset(wsrc, 0.0)
            for _w in range(N_WARMUP):
                w_ps = mmp.tile([128, 128], f32, tag="mm")
                nc.tensor.matmul(w_ps, wsrc, wsrc)

            xt_sb = big.tile([128, NTB, 4, TB], bf16)
            _xt_r = xt_in.ap()
            wqk_sb = big.tile([128, 4, 128], bf16)
            wv_sb = big.tile([128, 4, HS], bf16)
            wp_sb = big.tile([128, C], bf16)
            cc_sb = big.tile([128, T], bf16)
            ss_sb = big.tile([128, T], bf16)

            # inputs issued on sync in consumption order; xt block 0 in
            # cn-chunks so the first matmul starts as early as possible
            nc.sync.dma_start(out=wqk_sb, in_=wqk_in.ap())
            for cn in range(4):
                nc.sync.dma_start(out=xt_sb[:, 0, cn], in_=_xt_r[:, 0, cn])
            # compact rope tables + on-chip row expansion
            nc.sync.dma_start(out=cc_sb[0:32, :], in_=ccc_in.ap())
            nc.sync.dma_start(out=cc_sb[32:64, :], in_=cc_sb[0:32, :])
            nc.sync.dma_start(out=cc_sb[64:128, :], in_=cc_sb[0:64, :])
            nc.sync.dma_start(out=ss_sb[0:64, :], in_=ssc_in.ap())
            nc.sync.dma_start(out=ss_sb[64:128, :], in_=ss_sb[0:64, :])
            nc.gpsimd.dma_start(out=wv_sb, in_=wv_in.ap())
            nc.gpsimd.dma_start(out=wp_sb, in_=wp_in.ap())
            nc.sync.dma_start(out=xt_sb[:, 1], in_=_xt_r[:, 1])
            nc.sync.dma_start(out=xt_sb[:, 2], in_=_xt_r[:, 2])

            qkr = big.tile([128, T], bf16)    # rows 0:64 q_rot^T, 64:128 k_rot^T
            krqr = big.tile([128, T], bf16)   # rows 0:64 k_rot^T, 64:128 q_rot^T
            v_ones = big.tile([128, NJC, HS + 1], bf16)
            nc.vector.memset(v_ones[:, :, HS], 1.0)

            out_pending = []  # deferred (ot, yt_sb, i0) output DMAs
            cproj_pending = []  # deferred (yt_sb, ytd, i0) c_proj emissions

            def flush_cproj(last=False):
                while cproj_pending:
                    yts_, ytd_, i0_ = cproj_pending.pop(0)
                    ot = outp.tile([128, 4, TB], bf16, tag="ot")
                    for qp in range(2):
                        qa, qb = 2 * qp, 2 * qp + 1
                        if last:
                            # borrow free S^T PSUM banks: no WAR stall
                            stt = stp.tile([128, 2 * TB], f32, tag="st")
                            opa, opb = stt[:, 0:TB], stt[:, TB:2 * TB]
                        else:
                            opa = mmp.tile([128, TB], f32, tag="mm")
                            opb = mmp.tile([128, TB], f32, tag="mm")
                        nc.tensor.matmul(opa, yts_[0:HS, qa * 128:(qa + 1) * 128],
                                         wp_sb[0:HS, :], tile_position=(0, 0))
                        nc.tensor.matmul(opb, ytd_[64:128, qb * 128:(qb + 1) * 128],
                                         wp_sb[64:128, :], tile_position=(64, 0))
                        nc.scalar.copy(ot[:, qa, :], opa)
                        nc.vector.tensor_copy(ot[:, qb, :], opb)
                        if last:
                            nc.scalar.dma_start(out=out_u.ap()[:, i0_ // TB, qa:qb + 1],
                                                in_=ot[:, qa:qb + 1, :])
                    if last:
                        nc.scalar.dma_start(out=cs_out.ap()[0:1, i0_:i0_ + TB],
                                            in_=yts_[HS:HS + 1, :])
                    else:
                        out_pending.append((ot, yts_, i0_))

            def flush_out():
                while out_pending:
                    ot_, yts_, i0_ = out_pending.pop(0)
                    nc.sync.dma_start(out=out_u.ap()[:, i0_ // TB], in_=ot_)
                    nc.sync.dma_start(out=cs_out.ap()[0:1, i0_:i0_ + TB],
                                      in_=yts_[HS:HS + 1, :])

            def phase_a(tb):
                flush_out()
                tc0 = tb * TB
                if tb + 3 < NTB:  # distance-2 prefetch (tb+1, tb+2 already queued)
                    nc.sync.dma_start(out=xt_sb[:, tb + 3], in_=_xt_r[:, tb + 3])
                qk_ps = mmp.tile([128, TB], f32, tag="mm")
                for cn in range(4):
                    nc.tensor.matmul(qk_ps, wqk_sb[:, cn, :], xt_sb[:, tb, cn, :],
                                     start=(cn == 0), stop=(cn == 3))
                qk_sb = qkb.tile([128, TB], bf16, tag="qk")
                nc.vector.tensor_copy(qk_sb, qk_ps)
                # pair-swap via contiguous 32-row half swaps (head dims are
                # host-reordered so RoPE pairs sit 32 apart)
                qks_sb = qksb.tile([128, TB], bf16, tag="qks")
                nc.gpsimd.dma_start(out=qks_sb[0:32, :], in_=qk_sb[32:64, :])
                nc.gpsimd.dma_start(out=qks_sb[32:64, :], in_=qk_sb[0:32, :])
                nc.gpsimd.dma_start(out=qks_sb[64:96, :], in_=qk_sb[96:128, :])
                nc.gpsimd.dma_start(out=qks_sb[96:128, :], in_=qk_sb[64:96, :])
                # all-bf16 RoPE combine: every DVE op runs in 2x mode
                t1 = ropet.tile([128, TB], bf16, tag="t1")
                nc.vector.tensor_mul(t1, qks_sb, ss_sb[:, tc0:tc0 + TB])
                t2 = ropet.tile([128, TB], bf16, tag="t2")
                nc.vector.tensor_mul(t2, qk_sb, cc_sb[:, tc0:tc0 + TB])
                nc.vector.tensor_add(qkr[:, tc0:tc0 + TB], t2, t1)
                # swapped half-duplicate for the row-paired S^T matmuls
                nc.sync.dma_start(out=krqr[0:64, tc0:tc0 + TB], in_=qkr[64:128, tc0:tc0 + TB])
                nc.sync.dma_start(out=krqr[64:128, tc0:tc0 + TB], in_=qkr[0:64, tc0:tc0 + TB])
                # v in (t, d) layout: 4 t-chunks into one packed PSUM tile
                v_ps = vpp.tile([128, 4, HS], f32, tag="vp")
                for t4 in range(4):
                    for cn in range(4):
                        nc.tensor.matmul(v_ps[:, t4, :],
                                         xt_sb[:, tb, cn, t4 * 128:(t4 + 1) * 128],
                                         wv_sb[:, cn, :],
                                         start=(cn == 0), stop=(cn == 3))
                nc.vector.tensor_copy(v_ones[:, 4 * tb:4 * tb + 4, 0:HS], v_ps)

            def phase_b(ib):
                i0 = ib * TB
                nj = 4 * ib + 4
                flush_cproj()
                yt_ps = ytp.tile([128, TB], f32, tag="yt")
                pend = []  # (pt, col_base, j) awaiting their PV matmul

                def flush_pv(n):
                    while len(pend) > n:
                        pt_, cb_, j_ = pend.pop(0)
                        v0_ = max(0, j_ * JC - i0)
                        nc.tensor.matmul(yt_ps[0:HS + 1, v0_:TB], v_ones[:, j_, :],
                                         pt_[:, cb_ + v0_:cb_ + TB],
                                         start=(j_ == 0), stop=(j_ == nj - 1),
                                         skip_group_check=True)

                def emit_st(m):
                    j_e, j_o = 2 * m, 2 * m + 1
                    ve = max(0, j_e * JC - i0)
                    vo = max(0, j_o * JC - i0)
                    st = stp.tile([128, 2 * TB], f32, tag="st")
                    nc.tensor.matmul(st[:, ve:TB], krqr[0:64, j_e * JC:(j_e + 1) * JC],
                                     qkr[0:64, i0 + ve:i0 + TB], tile_position=(0, 0))
                    nc.tensor.matmul(st[:, TB + vo:2 * TB],
                                     qkr[64:128, j_o * JC:(j_o + 1) * JC],
                                     krqr[64:128, i0 + vo:i0 + TB], tile_position=(64, 0))
                    return st, j_e, j_o, ve, vo

                def emit_exp(m, st, j_e, j_o, ve, vo):
                    pt = ptp.tile([128, 2 * TB], bf16, tag="pt")
                    if m % DVE_EXP_FRAC == 1:  # spread some pairs onto DVE
                        nc.vector.tensor_scalar(pt[:, ve:2 * TB].bitcast(u16),
                                                st[:, ve:2 * TB], SCH_A, SCH_B,
                                                Mult, Add)
                    else:
                        nc.scalar.activation(pt[:, ve:2 * TB], st[:, ve:2 * TB],
                                             Exp, scale=0.125)
                    for cb, j, v0 in ((0, j_e, ve), (TB, j_o, vo)):
                        if j * JC + JC - 1 > i0:  # diagonal band elementwise mask
                            b0, b1 = v0, min(TB, v0 + JC)
                            nc.gpsimd.affine_select(
                                out=pt[:, cb + b0:cb + b1], in_=pt[:, cb + b0:cb + b1],
                                compare_op=mybir.AluOpType.is_ge,
                                fill=0.0, base=i0 + b0 - j * JC,
                                pattern=[[1, b1 - b0]], channel_multiplier=-1)
                        pend.append((pt, cb, j))

                # emit S^T pairs in groups of two so next-pair LDWEIGHTS
                # overlaps the in-flight quadrant matmuls
                ms = list(range(nj // 2))
                for g in range(0, len(ms), 2):
                    grp = [emit_st(m) for m in ms[g:g + 2]]
                    for m, args in zip(ms[g:g + 2], grp):
                        emit_exp(m, *args)
                        flush_pv(6)  # keep 3 pairs in flight
                flush_pv(0)

                yt_sb = ytsb.tile([128, TB], bf16, tag="yts")
                nc.vector.tensor_copy(yt_sb[0:HS + 1, :], yt_ps[0:HS + 1, :])
                # duplicate y^T into rows 64:128 for the c_proj quadrant pair
                ytd = ytdp.tile([128, TB], bf16, tag="ytd")
                nc.sync.dma_start(out=ytd[64:128, :], in_=yt_sb[0:64, :])
                cproj_pending.append((yt_sb, ytd, i0))
                if ib == NTB - 1:
                    flush_cproj(last=True)

            phase_a(0)
            for ib in range(NTB):
                if ib + 1 < NTB:
                    phase_a(ib + 1)
                phase_b(ib)
            flush_out()

    _legalize_waits(nc)
    return nc


_cached = {}


def _get_nc():
    if "nc" not in _cached:
        _cached["nc"] = _build_nc()
    return _cached["nc"]


def _prep_inputs(x, rope, W_attn, W_proj):
    bf16 = ml_dtypes.bfloat16
    # (C, T) -> [p, tb, n, t] so per-partition DMA runs are 4KB
    xt = np.ascontiguousarray(
        x[0].T.reshape(4, 128, NTB, TB).transpose(1, 2, 0, 3)).astype(bf16)
    cos = np.asarray(rope[..., 0], dtype=np.float32)        # (T, HS//2)
    sin = np.asarray(rope[..., 1], dtype=np.float32)
    # pair-reordered rope tables: head dim d=2f -> row f, d=2f+1 -> row 32+f
    ccc = np.ascontiguousarray(cos.T).astype(bf16)          # (32, T)
    ssc = np.ascontiguousarray(
        np.concatenate([-sin.T, sin.T], axis=0)).astype(bf16)  # (64, T)

    Wa = np.asarray(W_attn, dtype=np.float32)
    Wp = np.asarray(W_proj, dtype=np.float32)
    # evens-then-odds permutation of the 64 head dims
    perm = np.concatenate([np.arange(0, HS, 2), np.arange(1, HS, 2)])

    in_maps = []
    for h in range(NCORES):
        Wq = Wa[h * HS:(h + 1) * HS][perm]                  # (HS, C) reordered
        Wk = Wa[C + h * HS:C + (h + 1) * HS][perm]
        Wv = Wa[2 * C + h * HS:2 * C + (h + 1) * HS]
        wqk = np.concatenate([Wq.T, Wk.T], axis=1)          # (C, 128)
        # prepack to [128, 4, 128] so per-partition DMA runs are 1KB
        wqk = np.ascontiguousarray(
            wqk.reshape(4, 128, 128).transpose(1, 0, 2)).astype(bf16)
        wv = np.ascontiguousarray(
            Wv.T.reshape(4, 128, HS).transpose(1, 0, 2)).astype(bf16)
        wp1 = Wp[:, h * HS:(h + 1) * HS].T.astype(bf16)               # (HS, C)
        wp = np.ascontiguousarray(np.concatenate([wp1, wp1], axis=0))  # dup for quadrant pair
        in_maps.append({
            "xt": xt, "wqk": wqk, "wv": wv, "wp": wp, "ccc": ccc, "ssc": ssc,
        })
    return in_maps


def run_cores(x, rope, W_attn, W_proj, trace=False):
    """Returns BassKernelResults over the 8 cores."""
    nc = _get_nc()
    in_maps = _prep_inputs(x, rope, W_attn, W_proj)
    res = run_bass_kernel_spmd(nc, in_maps, list(range(NCORES)), trace=trace)
    return res


def kernel(x, rope, mask, W_attn, W_proj):
    res = run_cores(x, rope, W_attn, W_proj, trace=False)
    out = np.zeros((T, C), dtype=np.float32)
    for h in range(NCORES):
        r = res.results[h]
        cs = np.asarray(r["cs"], dtype=np.float32).reshape(T, 1)
        ou = np.asarray(r["out_u"], dtype=np.float32).transpose(1, 2, 0, 3).reshape(T, C)
        out += ou / cs
    return out.reshape(B, T, C).astype(np.float32)
